# revision 4
# baseline (speedup 1.0000x reference)
"""Trainium2 Bass kernel for nn_Middle_Moudle_v3_ablation_meta.

Computes, per episode (b,s) pair:
  - adaptive avg pool of support/query images [640,19,19] -> [640,5,5]
  - meta_input = concat([s_pooled, q_pooled], channel axis)
  - all-pairs cosine similarity over channels between support/query spatial
    positions, then max over query positions -> [25]

Strategy (8 NeuronCores, data parallel over the 100 (b,s) episodes):
  - Host: restage inputs as [ep, img, spatial(384 padded), C] fp32 so the
    361 spatial positions sit on SBUF partitions (3 chunks of 128).
  - Device: pooling over BOTH spatial axes is a single PE matmul per
    (img, c-chunk) against a precomputed [361,25] Kronecker pooling matrix
    (stationary = image chunk, so output lands as [c_chunk, 25] directly).
  - dots/norms via tiny PE matmuls contracting over channels; cosine
    normalization + max on DVE/ACT.
"""

import os
import numpy as np

N_WAY = 5
K_SHOT = 1
BATCH = 4
B = N_WAY * BATCH          # 20
S = N_WAY * K_SHOT         # 5
C = 640
W = 19
NW = 5
P = NW * NW                # 25
SP = W * W                 # 361
SP_PAD = 384               # 3 * 128
KC = 3                     # spatial partition chunks
MC = 5                     # channel chunks of 128
EPS = 1e-8

N_CORES = 8
EP_TOT = B * S             # 100
EPC = 13                   # episode slots per core (padded)
COUNTS = [13, 13, 13, 13, 12, 12, 12, 12]
STARTS = np.cumsum([0] + COUNTS).tolist()

_CACHE = {}


def _pool_matrix():
    """[SP_PAD, 25] fp32: PW[r*19+t, i*5+j] = A[i,r]*A[j,t], zero pad rows."""
    A = np.zeros((NW, W), dtype=np.float64)
    for i in range(NW):
        s = (i * W) // NW
        e = -((-(i + 1) * W) // NW)
        A[i, s:e] = 1.0 / (e - s)
    pw = np.zeros((SP_PAD, P), dtype=np.float64)
    for i in range(NW):
        for j in range(NW):
            pw[:SP, i * NW + j] = np.outer(A[i], A[j]).reshape(SP)
    return pw.astype(np.float32)


def _build_program(reps=None):
    """Build the SPMD program. reps=None -> production (one pass).
    reps=R -> wrap the whole body in a dynamic For_i loop for HW timing."""
    import concourse.bass as bass
    import concourse.tile as tile
    from concourse import bacc, mybir
    from concourse.bass_utils import axon_active

    f32 = mybir.dt.float32

    nc = bacc.Bacc(
        "TRN2",
        target_bir_lowering=False,
        debug=False,
        enable_asserts=False,
        num_devices=N_CORES,
    )

    x = nc.dram_tensor("x", [EPC, 2, KC, 128, C], f32, kind="ExternalInput").ap()
    pw = nc.dram_tensor("pw", [KC, 128, P], f32, kind="ExternalInput").ap()
    ident = nc.dram_tensor("ident", [P, P], f32, kind="ExternalInput").ap()
    meta = nc.dram_tensor("meta", [EPC, 2, MC, 128, P], f32, kind="ExternalOutput").ap()
    osim = nc.dram_tensor("osim", [EPC, P], f32, kind="ExternalOutput").ap()

    with tile.TileContext(nc) as tc:
        from contextlib import ExitStack

        with ExitStack() as ctx:
            const_pool = ctx.enter_context(tc.tile_pool(name="const", bufs=1))
            xpool = ctx.enter_context(tc.tile_pool(name="xp", bufs=3))
            stage = ctx.enter_context(tc.tile_pool(name="stage", bufs=3))
            small = ctx.enter_context(tc.tile_pool(name="small", bufs=2))
            ypsum = ctx.enter_context(tc.tile_pool(name="ypsum", bufs=3, space="PSUM"))
            spsum = ctx.enter_context(tc.tile_pool(name="spsum", bufs=1, space="PSUM"))

            pw_sb = const_pool.tile([128, KC, P], f32)
            nc.sync.dma_start(pw_sb[:], pw.rearrange("kc p n -> p kc n"))
            ident_sb = const_pool.tile([P, P], f32)
            nc.sync.dma_start(ident_sb[:], ident)
            ones_sb = const_pool.tile([128, P], f32)
            nc.vector.memset(ones_sb[:], 1.0)
            ocols = const_pool.tile([P, 16], f32)

            def body():
                _episodes(nc, tc, mybir, f32, x, meta, osim, pw_sb, ident_sb,
                          ones_sb, ocols, xpool, stage, small, ypsum, spsum)

            if reps is None:
                body()
            else:
                with tc.For_i(
                    0, reps, 1,
                    hint_engines=(
                        mybir.EngineType.PE,
                        mybir.EngineType.Activation,
                        mybir.EngineType.DVE,
                    ),
                ):
                    body()

    nc.compile()
    return nc


def _episodes(nc, tc, mybir, f32, x, meta, osim, pw_sb, ident_sb, ones_sb,
              ocols, xpool, stage, small, ypsum, spsum):
    if True:
        if True:
            for e in range(EPC):
                xt = xpool.tile([128, 2, KC, C], f32)
                nc.sync.dma_start(xt[:], x[e].rearrange("img kc p c -> p img kc c"))

                mstage = stage.tile([128, 2, MC, P], f32)
                for img in range(2):
                    for mc in range(MC):
                        yp = ypsum.tile([128, P], f32)
                        for kc in range(KC):
                            nc.tensor.matmul(
                                yp[:],
                                xt[:, img, kc, mc * 128:(mc + 1) * 128],
                                pw_sb[:, kc, :],
                                start=(kc == 0),
                                stop=(kc == KC - 1),
                            )
                        nc.scalar.copy(mstage[:, img, mc, :], yp[:])

                nc.scalar.dma_start(
                    meta[e].rearrange("img mc p n -> p img mc n"), mstage[:]
                )

                ysq = stage.tile([128, 2, MC, P], f32)
                nc.vector.tensor_mul(ysq[:], mstage[:], mstage[:])

                dots = spsum.tile([P, P], f32)
                for mc in range(MC):
                    nc.tensor.matmul(
                        dots[:],
                        mstage[:, 0, mc, :],
                        mstage[:, 1, mc, :],
                        start=(mc == 0),
                        stop=(mc == MC - 1),
                    )
                sn2 = spsum.tile([P, 1], f32)
                for mc in range(MC):
                    nc.tensor.matmul(
                        sn2[:],
                        ysq[:, 0, mc, :],
                        ones_sb[:, 0:1],
                        start=(mc == 0),
                        stop=(mc == MC - 1),
                    )
                qn2r = spsum.tile([P, P], f32)
                for mc in range(MC):
                    nc.tensor.matmul(
                        qn2r[:],
                        ones_sb[:],
                        ysq[:, 1, mc, :],
                        start=(mc == 0),
                        stop=(mc == MC - 1),
                    )

                sn = small.tile([P, 1], f32)
                nc.scalar.sqrt(sn[:], sn2[:])
                qnr = small.tile([P, P], f32)
                nc.scalar.sqrt(qnr[:], qn2r[:])
                den = small.tile([P, P], f32)
                nc.vector.tensor_scalar_mul(den[:], qnr[:], sn[:, 0:1])
                nc.vector.tensor_scalar_max(den[:], den[:], EPS)
                rec = small.tile([P, P], f32)
                nc.vector.reciprocal(rec[:], den[:])
                sim = small.tile([P, P], f32)
                nc.vector.tensor_mul(sim[:], dots[:], rec[:])
                nc.vector.reduce_max(
                    ocols[:, e:e + 1], sim[:], axis=mybir.AxisListType.X
                )

            tp = spsum.tile([EPC, P], f32)
            nc.tensor.transpose(tp[:], ocols[:, 0:EPC], ident_sb[:])
            ofin = small.tile([EPC, P], f32)
            nc.scalar.copy(ofin[:], tp[:])
            nc.scalar.dma_start(osim[:], ofin[:])


def _get_program():
    if "nc" not in _CACHE:
        _CACHE["nc"] = _build_program()
        _CACHE["pw"] = _pool_matrix().reshape(KC, 128, P)
        _CACHE["ident"] = np.eye(P, dtype=np.float32)
    return _CACHE["nc"], _CACHE["pw"], _CACHE["ident"]


def _stage_inputs(support_x, query_x):
    """Build per-core input arrays [EPC, 2, KC, 128, C]."""
    sx = np.ascontiguousarray(support_x, dtype=np.float32).reshape(EP_TOT, C, SP)
    qx = np.ascontiguousarray(query_x, dtype=np.float32).reshape(EP_TOT, C, SP)
    xs = np.zeros((N_CORES, EPC, 2, SP_PAD, C), dtype=np.float32)
    for c in range(N_CORES):
        n = COUNTS[c]
        sl = slice(STARTS[c], STARTS[c] + n)
        xs[c, :n, 0, :SP, :] = sx[sl].transpose(0, 2, 1)
        xs[c, :n, 1, :SP, :] = qx[sl].transpose(0, 2, 1)
        if n < EPC:
            xs[c, n:, :, :SP, :] = 1.0
    return xs.reshape(N_CORES, EPC, 2, KC, 128, C)


def kernel(support_x, query_x):
    from concourse.bass_utils import run_bass_kernel_spmd

    nc, pw, ident = _get_program()
    xs = _stage_inputs(support_x, query_x)
    in_maps = [
        {"x": xs[c], "pw": pw, "ident": ident} for c in range(N_CORES)
    ]
    res = run_bass_kernel_spmd(nc, in_maps, core_ids=list(range(N_CORES)))

    meta = np.empty((EP_TOT, 2 * C, P), dtype=np.float32)
    out = np.empty((EP_TOT, P), dtype=np.float32)
    for c in range(N_CORES):
        n = COUNTS[c]
        sl = slice(STARTS[c], STARTS[c] + n)
        meta[sl] = res.results[c]["meta"].reshape(EPC, 2 * C, P)[:n]
        out[sl] = res.results[c]["osim"][:n]

    meta = meta.reshape(B, S, 2 * C, NW, NW)
    out = out.reshape(B, S, P)
    return meta, out
